# revision 6
# baseline (speedup 1.0000x reference)
"""Trainium2 Bass kernel for nn_AdaptiveRouterMultiStep (8-core data parallel).

Math: per sample, per step t:
    scores_t = x_t @ wkq              (wkq = fold of Wk and q)
    attn_t   = softmax_l(scores_t)    per head
    logits_t = sum_{h,l} attn_t[h,l] * (x_t @ Mh[h])      (Mh = Wv_h @ Wmlp_h)
    w_t      = softmax(logits_t / T)  (straight-through: forward uses full softmax)
    W_t      = sum_k w_t[k] * Wb[k]
    x_{t+1}  = diag(1 + L*a*mean_h attn_t) @ x_t @ W_t

Row-scaling commutes with right matmuls, so with D_t = prod(1+s_tau) and
What_t = W_1...W_t:  x_{t+1} = D_t * (x_1 @ What_t).  All heavy compute is
x_1 against small matrices: scores passes (N=2 moving), attn@x_1 passes
(N=2 moving, PSUM-accumulated), and one final N=128 pass. x_1 is kept in
SBUF in both LC ([l,c], l on partitions) and CL (transposed) bf16 layouts.
"""

import numpy as np

import concourse.bass as bass
import concourse.bacc as bacc
import concourse.bass_isa as bass_isa
import concourse.tile as tile
from concourse import mybir
from concourse.bass_utils import run_bass_kernel_spmd

F32 = mybir.dt.float32
BF16 = mybir.dt.bfloat16

B, H, W, C = 16, 112, 112, 128
L = H * W            # 12544
G = L // 128         # 98 chunks of 128 rows
NCORES = 8
BLOC = B // NCORES   # 2 samples per core
STEPS = 3
T_ROUTE = 1.5
LG = 7               # load-group: chunks per input DMA group (14 groups)
FG = 4               # final-group: chunks per output PSUM group


def _rows(ap, s, c0, nch):
    """DRAM view of rows [s*L + c0*128, +nch*128) as [128(p), nch, C]."""
    r0 = s * L + c0 * 128
    return ap[r0:r0 + nch * 128, :].rearrange("(n p) c -> p n c", p=128)


def build(la2: float):
    nc = bacc.Bacc(None, target_bir_lowering=False)

    x_ext = nc.declare_dram_parameter("x", [BLOC * L, C], F32, isOutput=False)
    wkq_ext = nc.declare_dram_parameter("wkq", [C, 2], F32, isOutput=False)
    mh_ext = nc.declare_dram_parameter("mh", [C, 8], F32, isOutput=False)
    wb_ext = nc.declare_dram_parameter("wb", [C, 4 * C], F32, isOutput=False)
    id_ext = nc.declare_dram_parameter("ident", [C, C], F32, isOutput=False)
    out_ext = nc.declare_dram_parameter("out", [BLOC * L, C], F32, isOutput=True)

    with tile.TileContext(nc) as tc:
        with (
            tc.tile_pool(name="persist", bufs=1) as pp,
            tc.tile_pool(name="stage", bufs=4) as stg,
            tc.tile_pool(name="small", bufs=4) as sm,
            tc.tile_pool(name="ebuf", bufs=4) as ep,
            tc.tile_pool(name="ps_scores", bufs=2, space="PSUM") as pscore,
            tc.tile_pool(name="ps_small", bufs=3, space="PSUM") as psm,
            tc.tile_pool(name="ps_fin", bufs=2, space="PSUM") as pfin,
        ):
            # ---- constants / weights ----
            wkq_f = pp.tile([C, 2], F32, tag="wkq_f")
            nc.sync.dma_start(wkq_f, wkq_ext[:, :])
            mh_f = pp.tile([C, 8], F32, tag="mh_f")
            nc.sync.dma_start(mh_f, mh_ext[:, :])
            wb_f = pp.tile([C, C, 4], F32, tag="wb_f")  # k innermost
            nc.sync.dma_start(wb_f, wb_ext[:, :].rearrange("c (d k) -> c d k", k=4))
            id_f = pp.tile([C, C], F32, tag="id_f")
            nc.sync.dma_start(id_f, id_ext[:, :])

            wkq_b = pp.tile([C, 2], BF16, tag="wkq_b")
            nc.vector.tensor_copy(wkq_b, wkq_f)
            mh_b = pp.tile([C, 8], BF16, tag="mh_b")
            nc.vector.tensor_copy(mh_b, mh_f)
            id_b = pp.tile([C, C], BF16, tag="id_b")
            nc.vector.tensor_copy(id_b, id_f)

            # ---- per-sample persistent state ----
            x_lc = [pp.tile([128, G, C], BF16, tag=f"x_lc{s}", name=f"x_lc{s}")
                    for s in range(BLOC)]
            x_cl = [pp.tile([128, L], BF16, tag=f"x_cl{s}", name=f"x_cl{s}")
                    for s in range(BLOC)]
            E = [pp.tile([128, G, 2], F32, tag=f"E{s}", name=f"E{s}")
                 for s in range(BLOC)]
            tmp2 = [pp.tile([128, G, 2], F32, tag=f"tmp2{s}", name=f"tmp2{s}")
                    for s in range(BLOC)]
            abuf = [pp.tile([128, G, 2], BF16, tag=f"abuf{s}", name=f"abuf{s}")
                    for s in range(BLOC)]
            Dt = [pp.tile([128, G], F32, tag=f"D{s}", name=f"D{s}")
                  for s in range(BLOC)]

            # ---- load + cast + transpose, finely pipelined ----
            for s in range(BLOC):
                for g in range(G // LG):
                    stage = stg.tile([128, LG, C], F32, tag="stage")
                    nc.sync.dma_start(stage, _rows(x_ext, s, g * LG, LG))
                    ceng = nc.vector if g % 2 == 0 else nc.gpsimd
                    ceng.tensor_copy(x_lc[s][:, g * LG:(g + 1) * LG, :], stage)
                    teng = nc.sync if g % 2 == 0 else nc.scalar
                    teng.dma_start_transpose(
                        out=x_cl[s][:, g * LG * 128:(g + 1) * LG * 128].rearrange(
                            "c (n l) -> c n l", n=LG),
                        in_=x_lc[s][:, g * LG:(g + 1) * LG, :],
                    )

            # ---- step-1 scores pass ----
            ps_sc = [None] * BLOC
            for s in range(BLOC):
                ps_sc[s] = pscore.tile([128, G, 2], F32, tag="scores", name="ps_sc")
                for n in range(G):
                    nc.tensor.matmul(
                        ps_sc[s][:, n, :],
                        x_cl[s][:, n * 128:(n + 1) * 128],
                        wkq_b,
                        start=True, stop=True,
                    )

            Vb = [None] * BLOC      # What_t^T  [c_{t+1}, c1] bf16
            Whb = [None] * BLOC     # What_t    [c1, c_{t+1}] bf16 (t=STEPS only)
            ub = [None] * BLOC      # What_t @ wkq  [c1, 2] bf16
            Jf = [mh_f] * BLOC      # What_{t-1} @ Mh  [c1, 8] f32
            rzb = [None] * BLOC
            wsb = [None] * BLOC

            for t in range(1, STEPS + 1):
                # --- phase 1: softmax path (DVE/ACT/gpsimd, no PE) ---
                for s in range(BLOC):
                    zcat = sm.tile([128, 2], F32, tag="zcat")
                    if t == 1:
                        src = ps_sc[s]
                    else:
                        nc.vector.tensor_mul(
                            tmp2[s], ps_sc[s],
                            Dt[s][:, :, None].broadcast_to([128, G, 2]))
                        src = tmp2[s]
                    for h in range(2):
                        nc.scalar.activation(
                            out=E[s][:, :, h], in_=src[:, :, h],
                            func=mybir.ActivationFunctionType.Exp,
                            accum_out=zcat[:, h:h + 1])
                    zall = sm.tile([128, 2], F32, tag="zall")
                    nc.gpsimd.partition_all_reduce(
                        zall, zcat, 128, bass_isa.ReduceOp.add)
                    rzb[s] = sm.tile([128, 2], F32, tag="rzb", name="rzb")
                    nc.vector.reciprocal(rzb[s], zall)

                    nc.vector.tensor_mul(
                        tmp2[s], E[s], rzb[s][:, None, :].broadcast_to([128, G, 2]))
                    ssum = sm.tile([128, G], F32, tag="ssum")
                    nc.vector.tensor_reduce(
                        ssum, tmp2[s], mybir.AxisListType.X, mybir.AluOpType.add)
                    if t == 1:
                        nc.vector.tensor_copy(abuf[s], tmp2[s])
                        nc.vector.tensor_scalar(
                            out=Dt[s], in0=ssum, scalar1=la2, scalar2=1.0,
                            op0=mybir.AluOpType.mult, op1=mybir.AluOpType.add)
                    else:
                        nc.vector.tensor_mul(
                            abuf[s], tmp2[s],
                            Dt[s][:, :, None].broadcast_to([128, G, 2]))
                        stmp = sm.tile([128, G], F32, tag="stmp")
                        nc.vector.tensor_scalar(
                            out=stmp, in0=ssum, scalar1=la2, scalar2=1.0,
                            op0=mybir.AluOpType.mult, op1=mybir.AluOpType.add)
                        nc.vector.tensor_mul(Dt[s], Dt[s], stmp)

                # --- phase 2: attnx passes (PE) ---
                ps_ax = [None] * BLOC
                for s in range(BLOC):
                    ps_ax[s] = psm.tile([128, 2], F32, tag="ps_small", name="ps_ax")
                    for n in range(G):
                        nc.tensor.matmul(
                            ps_ax[s], x_lc[s][:, n, :], abuf[s][:, n, :],
                            start=(n == 0), stop=(n == G - 1))

                # --- phase 3: routing (DVE/ACT/gpsimd on [128, .]) ---
                for s in range(BLOC):
                    axs = sm.tile([128, 2], F32, tag="axs")
                    nc.vector.tensor_copy(axs, ps_ax[s])
                    prod8 = sm.tile([128, 8], F32, tag="prod8")
                    nc.vector.tensor_mul(
                        prod8.rearrange("p (h k) -> p h k", h=2),
                        Jf[s].rearrange("p (h k) -> p h k", h=2),
                        axs[:, :, None].broadcast_to([128, 2, 4]))
                    lgb = sm.tile([128, 8], F32, tag="lgb")
                    nc.gpsimd.partition_all_reduce(
                        lgb, prod8, 128, bass_isa.ReduceOp.add)
                    logits = sm.tile([128, 4], F32, tag="logits")
                    nc.vector.tensor_reduce(
                        logits, lgb.rearrange("p (h k) -> p k h", h=2),
                        mybir.AxisListType.X, mybir.AluOpType.add)
                    m1 = sm.tile([128, 1], F32, tag="m1")
                    nc.vector.tensor_reduce(
                        m1, logits, mybir.AxisListType.X, mybir.AluOpType.max)
                    nb = sm.tile([128, 1], F32, tag="nb")
                    nc.vector.tensor_scalar_mul(nb, m1, -1.0 / T_ROUTE)
                    e4 = sm.tile([128, 4], F32, tag="e4")
                    nc.scalar.activation(
                        out=e4, in_=logits, func=mybir.ActivationFunctionType.Exp,
                        bias=nb, scale=1.0 / T_ROUTE)
                    ssc = sm.tile([128, 1], F32, tag="ssc")
                    nc.vector.tensor_reduce(
                        ssc, e4, mybir.AxisListType.X, mybir.AluOpType.add)
                    rs = sm.tile([128, 1], F32, tag="rs")
                    nc.vector.reciprocal(rs, ssc)
                    wsb[s] = sm.tile([128, 4], F32, tag="wsb", name="wsb")
                    nc.vector.tensor_mul(
                        wsb[s], e4, rs.to_broadcast((128, 4)))

                # --- phase 4: W_t build + chain matmuls ---
                for s in range(BLOC):
                    wprod = sm.tile([C, C, 4], F32, tag="wprod")
                    nc.vector.tensor_mul(
                        wprod, wb_f,
                        wsb[s][:, None, :].broadcast_to([C, C, 4]))
                    wtf = sm.tile([C, C], F32, tag="wtf")
                    nc.vector.tensor_reduce(
                        wtf, wprod, mybir.AxisListType.X, mybir.AluOpType.add)
                    wtb = sm.tile([C, C], BF16, tag="wtb")
                    nc.vector.tensor_copy(wtb, wtf)

                    if t == 1:
                        ps_v = psm.tile([C, C], F32, tag="ps_small")
                        nc.tensor.matmul(ps_v, wtb, id_b, start=True, stop=True)
                        Vb[s] = sm.tile([C, C], BF16, tag="vb", name="vb1")
                        nc.vector.tensor_copy(Vb[s], ps_v)
                        Whb[s] = wtb
                    elif t < STEPS:
                        ps_v = psm.tile([C, C], F32, tag="ps_small")
                        nc.tensor.matmul(ps_v, wtb, Vb[s], start=True, stop=True)
                        nvb = sm.tile([C, C], BF16, tag="vb")
                        nc.vector.tensor_copy(nvb, ps_v)
                        Vb[s] = nvb
                    else:
                        ps_w = psm.tile([C, C], F32, tag="ps_small")
                        nc.tensor.matmul(ps_w, Vb[s], wtb, start=True, stop=True)
                        nwh = sm.tile([C, C], BF16, tag="whb")
                        nc.vector.tensor_copy(nwh, ps_w)
                        Whb[s] = nwh

                    if t < STEPS:
                        ps_u = psm.tile([C, 2], F32, tag="ps_small")
                        nc.tensor.matmul(ps_u, Vb[s], wkq_b, start=True, stop=True)
                        nub = sm.tile([C, 2], BF16, tag="ub")
                        nc.vector.tensor_copy(nub, ps_u)
                        ub[s] = nub
                        ps_j = psm.tile([C, 8], F32, tag="ps_small")
                        nc.tensor.matmul(ps_j, Vb[s], mh_b, start=True, stop=True)
                        njf = sm.tile([C, 8], F32, tag="jf")
                        nc.vector.tensor_copy(njf, ps_j)
                        Jf[s] = njf

                # --- phase 5: next scores pass / final pass ---
                if t < STEPS:
                    for s in range(BLOC):
                        nsc = pscore.tile([128, G, 2], F32, tag="scores",
                                          name="ps_sc2")
                        for n in range(G):
                            nc.tensor.matmul(
                                nsc[:, n, :],
                                x_cl[s][:, n * 128:(n + 1) * 128],
                                ub[s], start=True, stop=True)
                        ps_sc[s] = nsc
                else:
                    ngroups = (G + FG - 1) // FG
                    for s in range(BLOC):
                        for g in range(ngroups):
                            nch = min(FG, G - g * FG)
                            ps_f = pfin.tile([128, FG, C], F32, tag="fin")
                            for j in range(nch):
                                n = g * FG + j
                                nc.tensor.matmul(
                                    ps_f[:, j, :],
                                    x_cl[s][:, n * 128:(n + 1) * 128],
                                    Whb[s], start=True, stop=True)
                            eb = ep.tile([128, FG, C], F32, tag="eb")
                            if g % 2 == 0:
                                nc.vector.tensor_mul(
                                    eb[:, :nch, :], ps_f[:, :nch, :],
                                    Dt[s][:, g * FG:g * FG + nch, None]
                                    .broadcast_to([128, nch, C]))
                            else:
                                for j in range(nch):
                                    nc.scalar.activation(
                                        out=eb[:, j, :], in_=ps_f[:, j, :],
                                        func=mybir.ActivationFunctionType.Copy,
                                        scale=Dt[s][:, g * FG + j:g * FG + j + 1])
                            nc.sync.dma_start(
                                _rows(out_ext, s, g * FG, nch), eb[:, :nch, :])
    nc.finalize()
    return nc


def kernel(x, Wk, Wv, q, Wmlp, Wb, alpha):
    x = np.asarray(x, np.float32)
    Wk = np.asarray(Wk, np.float32)
    Wv = np.asarray(Wv, np.float32)
    q = np.asarray(q, np.float32)
    Wmlp = np.asarray(Wmlp, np.float32)
    Wb = np.asarray(Wb, np.float32)
    a = float(np.log1p(np.exp(np.float64(np.asarray(alpha)))))
    la2 = L * a / 2.0

    heads, dh = q.shape
    wkq = (Wk.reshape(C, heads, dh) * q[None]).sum(-1) / np.sqrt(dh)
    wkq = np.ascontiguousarray(wkq, np.float32)                      # [C, 2]
    mh = np.zeros((C, heads, 4), np.float32)
    for h in range(heads):
        mh[:, h, :] = Wv[:, h * dh:(h + 1) * dh] @ Wmlp[h * dh:(h + 1) * dh, :]
    mh = np.ascontiguousarray(mh.reshape(C, 8))                      # [C, 8]
    wbc = np.ascontiguousarray(
        Wb.transpose(1, 2, 0).reshape(C, C * 4), np.float32)         # [C, (d k)]
    ident = np.eye(C, dtype=np.float32)

    nc = build(la2)
    xs = x.reshape(B, L, C)
    in_maps = []
    for i in range(NCORES):
        in_maps.append({
            "x": np.ascontiguousarray(
                xs[i * BLOC:(i + 1) * BLOC].reshape(BLOC * L, C)),
            "wkq": wkq, "mh": mh, "wb": wbc, "ident": ident,
        })
    res = run_bass_kernel_spmd(nc, in_maps, core_ids=list(range(NCORES)))
    outs = [res.results[i]["out"].reshape(BLOC, H, W, C) for i in range(NCORES)]
    return np.concatenate(outs, axis=0)


# revision 7
# speedup vs baseline: 1.5874x; 1.5874x over previous
"""Trainium2 Bass kernel for nn_AdaptiveRouterMultiStep (8-core data parallel).

Math: per sample, per step t:
    scores_t = x_t @ wkq              (wkq = fold of Wk and q)
    attn_t   = softmax_l(scores_t)    per head
    logits_t = sum_{h,l} attn_t[h,l] * (x_t @ Mh[h])      (Mh = Wv_h @ Wmlp_h)
    w_t      = softmax(logits_t / T)  (straight-through: forward uses full softmax)
    W_t      = sum_k w_t[k] * Wb[k]
    x_{t+1}  = diag(1 + L*a*mean_h attn_t) @ x_t @ W_t

Row-scaling commutes with right matmuls, so with D_t = prod(1+s_tau) and
What_t = W_1...W_t:  x_{t+1} = D_t * (x_1 @ What_t).  All heavy compute is
x_1 against small matrices: scores passes (N=2 moving), attn@x_1 passes
(N=2 moving, PSUM-accumulated), and one final N=128 pass. x_1 is kept in
SBUF in both LC ([l,c], l on partitions) and CL (transposed) bf16 layouts.
"""

import numpy as np

import concourse.bass as bass
import concourse.bacc as bacc
import concourse.bass_isa as bass_isa
import concourse.tile as tile
from concourse import mybir
from concourse.bass_utils import run_bass_kernel_spmd

F32 = mybir.dt.float32
BF16 = mybir.dt.bfloat16

B, H, W, C = 16, 112, 112, 128
L = H * W            # 12544
G = L // 128         # 98 chunks of 128 rows
NCORES = 8
BLOC = B // NCORES   # 2 samples per core
STEPS = 3
T_ROUTE = 1.5
LG = 14              # load-group: chunks per input DMA group (7 groups)
FG = 4               # final-group: chunks per output PSUM group


def _rows(ap, s, c0, nch):
    """DRAM view of rows [s*L + c0*128, +nch*128) as [128(p), nch, C]."""
    r0 = s * L + c0 * 128
    return ap[r0:r0 + nch * 128, :].rearrange("(n p) c -> p n c", p=128)


def build(la2: float):
    nc = bacc.Bacc(None, target_bir_lowering=False)

    x_ext = nc.declare_dram_parameter("x", [BLOC * L, C], F32, isOutput=False)
    wkq_ext = nc.declare_dram_parameter("wkq", [C, 2], F32, isOutput=False)
    mh_ext = nc.declare_dram_parameter("mh", [C, 8], F32, isOutput=False)
    wb_ext = nc.declare_dram_parameter("wb", [C, 4 * C], F32, isOutput=False)
    id_ext = nc.declare_dram_parameter("ident", [C, C], F32, isOutput=False)
    out_ext = nc.declare_dram_parameter("out", [BLOC * L, C], F32, isOutput=True)

    with tile.TileContext(nc) as tc:
        with (
            tc.tile_pool(name="persist", bufs=1) as pp,
            tc.tile_pool(name="stage", bufs=4) as stg,
            tc.tile_pool(name="small", bufs=4) as sm,
            tc.tile_pool(name="ebuf", bufs=4) as ep,
            tc.tile_pool(name="ps_scores", bufs=2, space="PSUM") as pscore,
            tc.tile_pool(name="ps_small", bufs=3, space="PSUM") as psm,
            tc.tile_pool(name="ps_fin", bufs=2, space="PSUM") as pfin,
        ):
            # ---- constants / weights ----
            wkq_f = pp.tile([C, 2], F32, tag="wkq_f")
            nc.sync.dma_start(wkq_f, wkq_ext[:, :])
            mh_f = pp.tile([C, 8], F32, tag="mh_f")
            nc.sync.dma_start(mh_f, mh_ext[:, :])
            wb_f = pp.tile([C, C, 4], F32, tag="wb_f")  # k innermost
            nc.sync.dma_start(wb_f, wb_ext[:, :].rearrange("c (d k) -> c d k", k=4))
            id_f = pp.tile([C, C], F32, tag="id_f")
            nc.sync.dma_start(id_f, id_ext[:, :])

            wkq_b = pp.tile([C, 2], BF16, tag="wkq_b")
            nc.vector.tensor_copy(wkq_b, wkq_f)
            mh_b = pp.tile([C, 8], BF16, tag="mh_b")
            nc.vector.tensor_copy(mh_b, mh_f)
            id_b = pp.tile([C, C], BF16, tag="id_b")
            nc.vector.tensor_copy(id_b, id_f)

            # ---- per-sample persistent state ----
            x_lc = [pp.tile([128, G, C], BF16, tag=f"x_lc{s}", name=f"x_lc{s}")
                    for s in range(BLOC)]
            x_cl = [pp.tile([128, L], BF16, tag=f"x_cl{s}", name=f"x_cl{s}")
                    for s in range(BLOC)]
            E = [pp.tile([128, G, 2], F32, tag=f"E{s}", name=f"E{s}")
                 for s in range(BLOC)]
            tmp2 = [pp.tile([128, G, 2], F32, tag=f"tmp2{s}", name=f"tmp2{s}")
                    for s in range(BLOC)]
            abuf = [pp.tile([128, G, 2], BF16, tag=f"abuf{s}", name=f"abuf{s}")
                    for s in range(BLOC)]
            Dt = [pp.tile([128, G], F32, tag=f"D{s}", name=f"D{s}")
                  for s in range(BLOC)]

            # ---- load + cast, then transpose (batched: xbar-mode flips
            # between DMA copy and DMA transpose serialize the engines) ----
            for s in range(BLOC):
                for g in range(G // LG):
                    stage = stg.tile([128, LG, C], F32, tag="stage")
                    nc.sync.dma_start(stage, _rows(x_ext, s, g * LG, LG))
                    nc.vector.tensor_copy(x_lc[s][:, g * LG:(g + 1) * LG, :], stage)
                for g in range(G // LG):
                    teng = nc.sync if g % 2 == 0 else nc.scalar
                    teng.dma_start_transpose(
                        out=x_cl[s][:, g * LG * 128:(g + 1) * LG * 128].rearrange(
                            "c (n l) -> c n l", n=LG),
                        in_=x_lc[s][:, g * LG:(g + 1) * LG, :],
                    )

            # ---- step-1 scores pass ----
            ps_sc = [None] * BLOC
            for s in range(BLOC):
                ps_sc[s] = pscore.tile([128, G, 2], F32, tag="scores", name="ps_sc")
                for n in range(G):
                    nc.tensor.matmul(
                        ps_sc[s][:, n, :],
                        x_cl[s][:, n * 128:(n + 1) * 128],
                        wkq_b,
                        start=True, stop=True,
                    )

            Vb = [None] * BLOC      # What_t^T  [c_{t+1}, c1] bf16
            Whb = [None] * BLOC     # What_t    [c1, c_{t+1}] bf16 (t=STEPS only)
            ub = [None] * BLOC      # What_t @ wkq  [c1, 2] bf16
            Jf = [mh_f] * BLOC      # What_{t-1} @ Mh  [c1, 8] f32
            rzb = [None] * BLOC
            wsb = [None] * BLOC

            for t in range(1, STEPS + 1):
                # --- phase 1: softmax path (DVE/ACT/gpsimd, no PE) ---
                for s in range(BLOC):
                    zcat = sm.tile([128, 2], F32, tag="zcat")
                    if t == 1:
                        src = ps_sc[s]
                    else:
                        nc.vector.tensor_mul(
                            tmp2[s], ps_sc[s],
                            Dt[s][:, :, None].broadcast_to([128, G, 2]))
                        src = tmp2[s]
                    for h in range(2):
                        nc.scalar.activation(
                            out=E[s][:, :, h], in_=src[:, :, h],
                            func=mybir.ActivationFunctionType.Exp,
                            accum_out=zcat[:, h:h + 1])
                    zall = sm.tile([128, 2], F32, tag="zall")
                    nc.gpsimd.partition_all_reduce(
                        zall, zcat, 128, bass_isa.ReduceOp.add)
                    rzb[s] = sm.tile([128, 2], F32, tag="rzb", name="rzb")
                    nc.vector.reciprocal(rzb[s], zall)

                    nc.vector.tensor_mul(
                        tmp2[s], E[s], rzb[s][:, None, :].broadcast_to([128, G, 2]))
                    ssum = sm.tile([128, G], F32, tag="ssum")
                    nc.vector.tensor_reduce(
                        ssum, tmp2[s], mybir.AxisListType.X, mybir.AluOpType.add)
                    if t == 1:
                        nc.vector.tensor_copy(abuf[s], tmp2[s])
                        nc.vector.tensor_scalar(
                            out=Dt[s], in0=ssum, scalar1=la2, scalar2=1.0,
                            op0=mybir.AluOpType.mult, op1=mybir.AluOpType.add)
                    else:
                        nc.vector.tensor_mul(
                            abuf[s], tmp2[s],
                            Dt[s][:, :, None].broadcast_to([128, G, 2]))
                        stmp = sm.tile([128, G], F32, tag="stmp")
                        nc.vector.tensor_scalar(
                            out=stmp, in0=ssum, scalar1=la2, scalar2=1.0,
                            op0=mybir.AluOpType.mult, op1=mybir.AluOpType.add)
                        nc.vector.tensor_mul(Dt[s], Dt[s], stmp)

                # --- phase 2: attnx passes (PE) ---
                ps_ax = [None] * BLOC
                for s in range(BLOC):
                    ps_ax[s] = psm.tile([128, 2], F32, tag="ps_small", name="ps_ax")
                    for n in range(G):
                        nc.tensor.matmul(
                            ps_ax[s], x_lc[s][:, n, :], abuf[s][:, n, :],
                            start=(n == 0), stop=(n == G - 1))

                # --- phase 3: routing (DVE/ACT/gpsimd on [128, .]) ---
                for s in range(BLOC):
                    axs = sm.tile([128, 2], F32, tag="axs")
                    nc.vector.tensor_copy(axs, ps_ax[s])
                    prod8 = sm.tile([128, 8], F32, tag="prod8")
                    nc.vector.tensor_mul(
                        prod8.rearrange("p (h k) -> p h k", h=2),
                        Jf[s].rearrange("p (h k) -> p h k", h=2),
                        axs[:, :, None].broadcast_to([128, 2, 4]))
                    lgb = sm.tile([128, 8], F32, tag="lgb")
                    nc.gpsimd.partition_all_reduce(
                        lgb, prod8, 128, bass_isa.ReduceOp.add)
                    logits = sm.tile([128, 4], F32, tag="logits")
                    nc.vector.tensor_reduce(
                        logits, lgb.rearrange("p (h k) -> p k h", h=2),
                        mybir.AxisListType.X, mybir.AluOpType.add)
                    m1 = sm.tile([128, 1], F32, tag="m1")
                    nc.vector.tensor_reduce(
                        m1, logits, mybir.AxisListType.X, mybir.AluOpType.max)
                    nb = sm.tile([128, 1], F32, tag="nb")
                    nc.vector.tensor_scalar_mul(nb, m1, -1.0 / T_ROUTE)
                    e4 = sm.tile([128, 4], F32, tag="e4")
                    nc.scalar.activation(
                        out=e4, in_=logits, func=mybir.ActivationFunctionType.Exp,
                        bias=nb, scale=1.0 / T_ROUTE)
                    ssc = sm.tile([128, 1], F32, tag="ssc")
                    nc.vector.tensor_reduce(
                        ssc, e4, mybir.AxisListType.X, mybir.AluOpType.add)
                    rs = sm.tile([128, 1], F32, tag="rs")
                    nc.vector.reciprocal(rs, ssc)
                    wsb[s] = sm.tile([128, 4], F32, tag="wsb", name="wsb")
                    nc.vector.tensor_mul(
                        wsb[s], e4, rs.to_broadcast((128, 4)))

                # --- phase 4: W_t build + chain matmuls ---
                for s in range(BLOC):
                    wprod = sm.tile([C, C, 4], F32, tag="wprod")
                    nc.vector.tensor_mul(
                        wprod, wb_f,
                        wsb[s][:, None, :].broadcast_to([C, C, 4]))
                    wtf = sm.tile([C, C], F32, tag="wtf")
                    nc.vector.tensor_reduce(
                        wtf, wprod, mybir.AxisListType.X, mybir.AluOpType.add)
                    wtb = sm.tile([C, C], BF16, tag="wtb")
                    nc.vector.tensor_copy(wtb, wtf)

                    if t == 1:
                        ps_v = psm.tile([C, C], F32, tag="ps_small")
                        nc.tensor.matmul(ps_v, wtb, id_b, start=True, stop=True)
                        Vb[s] = sm.tile([C, C], BF16, tag="vb", name="vb1")
                        nc.vector.tensor_copy(Vb[s], ps_v)
                        Whb[s] = wtb
                    elif t < STEPS:
                        ps_v = psm.tile([C, C], F32, tag="ps_small")
                        nc.tensor.matmul(ps_v, wtb, Vb[s], start=True, stop=True)
                        nvb = sm.tile([C, C], BF16, tag="vb")
                        nc.vector.tensor_copy(nvb, ps_v)
                        Vb[s] = nvb
                    else:
                        ps_w = psm.tile([C, C], F32, tag="ps_small")
                        nc.tensor.matmul(ps_w, Vb[s], wtb, start=True, stop=True)
                        nwh = sm.tile([C, C], BF16, tag="whb")
                        nc.vector.tensor_copy(nwh, ps_w)
                        Whb[s] = nwh

                    if t < STEPS:
                        ps_u = psm.tile([C, 2], F32, tag="ps_small")
                        nc.tensor.matmul(ps_u, Vb[s], wkq_b, start=True, stop=True)
                        nub = sm.tile([C, 2], BF16, tag="ub")
                        nc.vector.tensor_copy(nub, ps_u)
                        ub[s] = nub
                        ps_j = psm.tile([C, 8], F32, tag="ps_small")
                        nc.tensor.matmul(ps_j, Vb[s], mh_b, start=True, stop=True)
                        njf = sm.tile([C, 8], F32, tag="jf")
                        nc.vector.tensor_copy(njf, ps_j)
                        Jf[s] = njf

                # --- phase 5: next scores pass / final pass ---
                if t < STEPS:
                    for s in range(BLOC):
                        nsc = pscore.tile([128, G, 2], F32, tag="scores",
                                          name="ps_sc2")
                        for n in range(G):
                            nc.tensor.matmul(
                                nsc[:, n, :],
                                x_cl[s][:, n * 128:(n + 1) * 128],
                                ub[s], start=True, stop=True)
                        ps_sc[s] = nsc
                else:
                    ngroups = (G + FG - 1) // FG
                    for s in range(BLOC):
                        for g in range(ngroups):
                            nch = min(FG, G - g * FG)
                            ps_f = pfin.tile([128, FG, C], F32, tag="fin")
                            for j in range(nch):
                                n = g * FG + j
                                nc.tensor.matmul(
                                    ps_f[:, j, :],
                                    x_cl[s][:, n * 128:(n + 1) * 128],
                                    Whb[s], start=True, stop=True)
                            eb = ep.tile([128, FG, C], F32, tag="eb")
                            if g % 2 == 0:
                                nc.vector.tensor_mul(
                                    eb[:, :nch, :], ps_f[:, :nch, :],
                                    Dt[s][:, g * FG:g * FG + nch, None]
                                    .broadcast_to([128, nch, C]))
                            else:
                                for j in range(nch):
                                    nc.scalar.activation(
                                        out=eb[:, j, :], in_=ps_f[:, j, :],
                                        func=mybir.ActivationFunctionType.Copy,
                                        scale=Dt[s][:, g * FG + j:g * FG + j + 1])
                            nc.sync.dma_start(
                                _rows(out_ext, s, g * FG, nch), eb[:, :nch, :])
    nc.finalize()
    return nc


def kernel(x, Wk, Wv, q, Wmlp, Wb, alpha):
    x = np.asarray(x, np.float32)
    Wk = np.asarray(Wk, np.float32)
    Wv = np.asarray(Wv, np.float32)
    q = np.asarray(q, np.float32)
    Wmlp = np.asarray(Wmlp, np.float32)
    Wb = np.asarray(Wb, np.float32)
    a = float(np.log1p(np.exp(np.float64(np.asarray(alpha)))))
    la2 = L * a / 2.0

    heads, dh = q.shape
    wkq = (Wk.reshape(C, heads, dh) * q[None]).sum(-1) / np.sqrt(dh)
    wkq = np.ascontiguousarray(wkq, np.float32)                      # [C, 2]
    mh = np.zeros((C, heads, 4), np.float32)
    for h in range(heads):
        mh[:, h, :] = Wv[:, h * dh:(h + 1) * dh] @ Wmlp[h * dh:(h + 1) * dh, :]
    mh = np.ascontiguousarray(mh.reshape(C, 8))                      # [C, 8]
    wbc = np.ascontiguousarray(
        Wb.transpose(1, 2, 0).reshape(C, C * 4), np.float32)         # [C, (d k)]
    ident = np.eye(C, dtype=np.float32)

    nc = build(la2)
    xs = x.reshape(B, L, C)
    in_maps = []
    for i in range(NCORES):
        in_maps.append({
            "x": np.ascontiguousarray(
                xs[i * BLOC:(i + 1) * BLOC].reshape(BLOC * L, C)),
            "wkq": wkq, "mh": mh, "wb": wbc, "ident": ident,
        })
    res = run_bass_kernel_spmd(nc, in_maps, core_ids=list(range(NCORES)))
    outs = [res.results[i]["out"].reshape(BLOC, H, W, C) for i in range(NCORES)]
    return np.concatenate(outs, axis=0)


# revision 14
# speedup vs baseline: 1.6340x; 1.0294x over previous
"""Trainium2 Bass kernel for nn_AdaptiveRouterMultiStep (8-core data parallel).

Math: per sample, per step t:
    scores_t = x_t @ wkq              (wkq = fold of Wk and q)
    attn_t   = softmax_l(scores_t)    per head
    logits_t = sum_{h,l} attn_t[h,l] * (x_t @ Mh[h])      (Mh = Wv_h @ Wmlp_h)
    w_t      = softmax(logits_t / T)  (straight-through: forward uses full softmax)
    W_t      = sum_k w_t[k] * Wb[k]
    x_{t+1}  = diag(1 + L*a*mean_h attn_t) @ x_t @ W_t

Row-scaling commutes with right matmuls, so with D_t = prod(1+s_tau) and
What_t = W_1...W_t:  x_{t+1} = D_t * (x_1 @ What_t).  All heavy compute is
x_1 against small matrices: scores passes (N=2 moving), attn@x_1 passes
(N=2 moving, PSUM-accumulated), and one final N=128 pass. x_1 is kept in
SBUF in both LC ([l,c], l on partitions) and CL (transposed) bf16 layouts.
"""

import numpy as np

import concourse.bass as bass
import concourse.bacc as bacc
import concourse.bass_isa as bass_isa
import concourse.tile as tile
from concourse import mybir
from concourse.bass_utils import run_bass_kernel_spmd

F32 = mybir.dt.float32
BF16 = mybir.dt.bfloat16

B, H, W, C = 16, 112, 112, 128
L = H * W            # 12544
G = L // 128         # 98 chunks of 128 rows
NCORES = 8
BLOC = B // NCORES   # 2 samples per core
STEPS = 3
T_ROUTE = 1.5
LG = 14              # load-group: chunks per input DMA group (7 groups)
FG = 4               # final-group: chunks per output PSUM group


def _rows(ap, s, c0, nch):
    """DRAM view of rows [s*L + c0*128, +nch*128) as [128(p), nch, C]."""
    r0 = s * L + c0 * 128
    return ap[r0:r0 + nch * 128, :].rearrange("(n p) c -> p n c", p=128)


def build(la2: float):
    nc = bacc.Bacc(None, target_bir_lowering=False)

    x_ext = nc.declare_dram_parameter("x", [BLOC * L, C], F32, isOutput=False)
    wkq_ext = nc.declare_dram_parameter("wkq", [C, 2], F32, isOutput=False)
    mh_ext = nc.declare_dram_parameter("mh", [C, 8], F32, isOutput=False)
    wb_ext = nc.declare_dram_parameter("wb", [C, 4 * C], F32, isOutput=False)
    id_ext = nc.declare_dram_parameter("ident", [C, C], F32, isOutput=False)
    out_ext = nc.declare_dram_parameter("out", [BLOC * L, C], F32, isOutput=True)

    with tile.TileContext(nc) as tc:
        with (
            tc.tile_pool(name="persist", bufs=1) as pp,
            tc.tile_pool(name="stage", bufs=4) as stg,
            tc.tile_pool(name="small", bufs=4) as sm,
            tc.tile_pool(name="ebuf", bufs=4) as ep,
            tc.tile_pool(name="ps_scores", bufs=2, space="PSUM") as pscore,
            tc.tile_pool(name="ps_small", bufs=3, space="PSUM") as psm,
            tc.tile_pool(name="ps_fin", bufs=2, space="PSUM") as pfin,
        ):
            # ---- constants / weights ----
            wkq_f = pp.tile([C, 2], F32, tag="wkq_f")
            nc.sync.dma_start(wkq_f, wkq_ext[:, :])
            mh_f = pp.tile([C, 8], F32, tag="mh_f")
            nc.sync.dma_start(mh_f, mh_ext[:, :])
            wb_f = pp.tile([C, C, 4], F32, tag="wb_f")  # k innermost
            nc.sync.dma_start(wb_f, wb_ext[:, :].rearrange("c (d k) -> c d k", k=4))
            id_f = pp.tile([C, C], F32, tag="id_f")
            nc.sync.dma_start(id_f, id_ext[:, :])

            wkq_b = pp.tile([C, 2], BF16, tag="wkq_b")
            nc.vector.tensor_copy(wkq_b, wkq_f)
            mh_b = pp.tile([C, 8], BF16, tag="mh_b")
            nc.vector.tensor_copy(mh_b, mh_f)
            id_b = pp.tile([C, C], BF16, tag="id_b")
            nc.vector.tensor_copy(id_b, id_f)

            # ---- per-sample persistent state ----
            x_lc = [pp.tile([128, G, C], BF16, tag=f"x_lc{s}", name=f"x_lc{s}")
                    for s in range(BLOC)]
            x_cl = [pp.tile([128, L], BF16, tag=f"x_cl{s}", name=f"x_cl{s}")
                    for s in range(BLOC)]
            E = [pp.tile([128, G, 2], F32, tag=f"E{s}", name=f"E{s}")
                 for s in range(BLOC)]
            tmp2 = [pp.tile([128, G, 2], F32, tag=f"tmp2{s}", name=f"tmp2{s}")
                    for s in range(BLOC)]
            abuf = [pp.tile([128, G, 2], BF16, tag=f"abuf{s}", name=f"abuf{s}")
                    for s in range(BLOC)]
            Dt = [pp.tile([128, G], F32, tag=f"D{s}", name=f"D{s}")
                  for s in range(BLOC)]

            # ---- helpers emitted per sample (samples are deliberately
            # de-lockstepped: s0 computes its whole step chain while s1
            # ingests; s0's output DMA then overlaps s1's compute) ----
            ps_sc = [None] * BLOC
            Vb = [None] * BLOC      # What_t^T  [c_{t+1}, c1] bf16
            Whb = [None] * BLOC     # What_t    [c1, c_{t+1}] bf16 (t=STEPS only)
            ub = [None] * BLOC      # What_t @ wkq  [c1, 2] bf16
            Jf = [mh_f] * BLOC      # What_{t-1} @ Mh  [c1, 8] f32
            rzb = [None] * BLOC
            wsb = [None] * BLOC

            transposes = {0: [], 1: []}

            def emit_ingest(s):
                # batched loads then batched transposes: xbar-mode flips
                # between DMA copy and DMA transpose serialize the engines,
                # and concurrent copy-vs-transpose corrupts data (HW bug) --
                # explicitly order this sample's loads after the previous
                # sample's transposes
                for g in range(G // LG):
                    stage = stg.tile([128, LG, C], F32, tag="stage",
                                     name="stage")
                    li = nc.sync.dma_start(stage, _rows(x_ext, s, g * LG, LG))
                    if g == 0:
                        for other in range(BLOC):
                            if other != s:
                                for ti in transposes[other]:
                                    tile.add_dep_helper(
                                        li.ins, ti.ins, sync=True,
                                        reason="xbar copy-vs-transpose")
                    nc.vector.tensor_copy(
                        x_lc[s][:, g * LG:(g + 1) * LG, :], stage)
                for g in range(G // LG):
                    teng = nc.sync if g % 2 == 0 else nc.scalar
                    ti = teng.dma_start_transpose(
                        out=x_cl[s][:, g * LG * 128:(g + 1) * LG * 128].rearrange(
                            "c (n l) -> c n l", n=LG),
                        in_=x_lc[s][:, g * LG:(g + 1) * LG, :],
                    )
                    transposes[s].append(ti)

            def emit_scores1(s):
                ps_sc[s] = pscore.tile([128, G, 2], F32, tag="scores",
                                       name="ps_sc")
                for n in range(G):
                    nc.tensor.matmul(
                        ps_sc[s][:, n, :],
                        x_cl[s][:, n * 128:(n + 1) * 128],
                        wkq_b,
                        start=True, stop=True,
                    )

            def emit_steps(s):
                for t in range(1, STEPS + 1):
                    # --- softmax path (DVE/ACT/gpsimd, no PE) ---
                    zcat = sm.tile([128, 2], F32, tag="zcat")
                    if t == 1:
                        src = ps_sc[s]
                    else:
                        nc.vector.tensor_mul(
                            tmp2[s], ps_sc[s],
                            Dt[s][:, :, None].broadcast_to([128, G, 2]))
                        src = tmp2[s]
                    for h in range(2):
                        nc.scalar.activation(
                            out=E[s][:, :, h], in_=src[:, :, h],
                            func=mybir.ActivationFunctionType.Exp,
                            accum_out=zcat[:, h:h + 1])
                    zall = sm.tile([128, 2], F32, tag="zall")
                    nc.gpsimd.partition_all_reduce(
                        zall, zcat, 128, bass_isa.ReduceOp.add)
                    rzb[s] = sm.tile([128, 2], F32, tag="rzb", name="rzb")
                    nc.vector.reciprocal(rzb[s], zall)

                    nc.vector.tensor_mul(
                        tmp2[s], E[s], rzb[s][:, None, :].broadcast_to([128, G, 2]))
                    ssum = sm.tile([128, G], F32, tag="ssum")
                    nc.vector.tensor_reduce(
                        ssum, tmp2[s], mybir.AxisListType.X, mybir.AluOpType.add)
                    if t == 1:
                        nc.vector.tensor_copy(abuf[s], tmp2[s])
                        nc.vector.tensor_scalar(
                            out=Dt[s], in0=ssum, scalar1=la2, scalar2=1.0,
                            op0=mybir.AluOpType.mult, op1=mybir.AluOpType.add)
                    else:
                        nc.vector.tensor_mul(
                            abuf[s], tmp2[s],
                            Dt[s][:, :, None].broadcast_to([128, G, 2]))
                        stmp = sm.tile([128, G], F32, tag="stmp")
                        nc.vector.tensor_scalar(
                            out=stmp, in0=ssum, scalar1=la2, scalar2=1.0,
                            op0=mybir.AluOpType.mult, op1=mybir.AluOpType.add)
                        nc.vector.tensor_mul(Dt[s], Dt[s], stmp)

                    # --- attnx pass (PE) ---
                    ps_ax = psm.tile([128, 2], F32, tag="ps_small", name="ps_ax")
                    for n in range(G):
                        nc.tensor.matmul(
                            ps_ax, x_lc[s][:, n, :], abuf[s][:, n, :],
                            start=(n == 0), stop=(n == G - 1))

                    # --- routing (DVE/ACT/gpsimd on [128, .]) ---
                    axs = sm.tile([128, 2], F32, tag="axs")
                    nc.vector.tensor_copy(axs, ps_ax)
                    prod8 = sm.tile([128, 8], F32, tag="prod8")
                    nc.vector.tensor_mul(
                        prod8.rearrange("p (h k) -> p h k", h=2),
                        Jf[s].rearrange("p (h k) -> p h k", h=2),
                        axs[:, :, None].broadcast_to([128, 2, 4]))
                    lgb = sm.tile([128, 8], F32, tag="lgb")
                    nc.gpsimd.partition_all_reduce(
                        lgb, prod8, 128, bass_isa.ReduceOp.add)
                    logits = sm.tile([128, 4], F32, tag="logits")
                    nc.vector.tensor_reduce(
                        logits, lgb.rearrange("p (h k) -> p k h", h=2),
                        mybir.AxisListType.X, mybir.AluOpType.add)
                    m1 = sm.tile([128, 1], F32, tag="m1")
                    nc.vector.tensor_reduce(
                        m1, logits, mybir.AxisListType.X, mybir.AluOpType.max)
                    nb = sm.tile([128, 1], F32, tag="nb")
                    nc.vector.tensor_scalar_mul(nb, m1, -1.0 / T_ROUTE)
                    e4 = sm.tile([128, 4], F32, tag="e4")
                    nc.scalar.activation(
                        out=e4, in_=logits, func=mybir.ActivationFunctionType.Exp,
                        bias=nb, scale=1.0 / T_ROUTE)
                    ssc = sm.tile([128, 1], F32, tag="ssc")
                    nc.vector.tensor_reduce(
                        ssc, e4, mybir.AxisListType.X, mybir.AluOpType.add)
                    rs = sm.tile([128, 1], F32, tag="rs")
                    nc.vector.reciprocal(rs, ssc)
                    wsb[s] = sm.tile([128, 4], F32, tag="wsb", name="wsb")
                    nc.vector.tensor_mul(
                        wsb[s], e4, rs.to_broadcast((128, 4)))

                    # --- W_t build ---
                    wprod = sm.tile([C, C, 4], F32, tag="wprod")
                    nc.vector.tensor_mul(
                        wprod, wb_f,
                        wsb[s][:, None, :].broadcast_to([C, C, 4]))
                    wtf = sm.tile([C, C], F32, tag="wtf")
                    nc.vector.tensor_reduce(
                        wtf, wprod, mybir.AxisListType.X, mybir.AluOpType.add)
                    wtb = sm.tile([C, C], BF16, tag="wtb")
                    nc.vector.tensor_copy(wtb, wtf)

                    # --- chain matmuls ---
                    if t == 1:
                        ps_v = psm.tile([C, C], F32, tag="ps_small")
                        nc.tensor.matmul(ps_v, wtb, id_b, start=True, stop=True)
                        Vb[s] = sm.tile([C, C], BF16, tag="vb", name="vb1")
                        nc.vector.tensor_copy(Vb[s], ps_v)
                        Whb[s] = wtb
                    elif t < STEPS:
                        ps_v = psm.tile([C, C], F32, tag="ps_small")
                        nc.tensor.matmul(ps_v, wtb, Vb[s], start=True, stop=True)
                        nvb = sm.tile([C, C], BF16, tag="vb")
                        nc.vector.tensor_copy(nvb, ps_v)
                        Vb[s] = nvb
                    else:
                        ps_w = psm.tile([C, C], F32, tag="ps_small")
                        nc.tensor.matmul(ps_w, Vb[s], wtb, start=True, stop=True)
                        nwh = sm.tile([C, C], BF16, tag="whb")
                        nc.vector.tensor_copy(nwh, ps_w)
                        Whb[s] = nwh

                    if t < STEPS:
                        ps_u = psm.tile([C, 2], F32, tag="ps_small")
                        nc.tensor.matmul(ps_u, Vb[s], wkq_b, start=True, stop=True)
                        nub = sm.tile([C, 2], BF16, tag="ub")
                        nc.vector.tensor_copy(nub, ps_u)
                        ub[s] = nub
                        ps_j = psm.tile([C, 8], F32, tag="ps_small")
                        nc.tensor.matmul(ps_j, Vb[s], mh_b, start=True, stop=True)
                        njf = sm.tile([C, 8], F32, tag="jf")
                        nc.vector.tensor_copy(njf, ps_j)
                        Jf[s] = njf

                        nsc = pscore.tile([128, G, 2], F32, tag="scores",
                                          name="ps_sc2")
                        for n in range(G):
                            nc.tensor.matmul(
                                nsc[:, n, :],
                                x_cl[s][:, n * 128:(n + 1) * 128],
                                ub[s], start=True, stop=True)
                        ps_sc[s] = nsc
                    else:
                        # --- final pass + scaled eviction + output DMA ---
                        ngroups = (G + FG - 1) // FG
                        for g in range(ngroups):
                            nch = min(FG, G - g * FG)
                            ps_f = pfin.tile([128, FG, C], F32, tag="fin")
                            for j in range(nch):
                                n = g * FG + j
                                nc.tensor.matmul(
                                    ps_f[:, j, :],
                                    x_cl[s][:, n * 128:(n + 1) * 128],
                                    Whb[s], start=True, stop=True)
                            eb = ep.tile([128, FG, C], F32, tag="eb")
                            nc.vector.tensor_mul(
                                eb[:, :nch, :], ps_f[:, :nch, :],
                                Dt[s][:, g * FG:g * FG + nch, None]
                                .broadcast_to([128, nch, C]))
                            oi = nc.sync.dma_start(
                                _rows(out_ext, s, g * FG, nch), eb[:, :nch, :])
                            if g == 0:
                                # the xbar transpose <-> copy HW bug corrupts
                                # data when s1's ingest transposes overlap
                                # s0's output copies; order them explicitly
                                for other in range(BLOC):
                                    if other != s:
                                        for ti in transposes[other]:
                                            tile.add_dep_helper(
                                                oi.ins, ti.ins, sync=True,
                                                reason="xbar copy-vs-transpose")

            emit_ingest(0)
            emit_scores1(0)
            emit_ingest(1)
            emit_steps(0)
            emit_scores1(1)
            emit_steps(1)
    nc.finalize()
    return nc


def kernel(x, Wk, Wv, q, Wmlp, Wb, alpha):
    x = np.asarray(x, np.float32)
    Wk = np.asarray(Wk, np.float32)
    Wv = np.asarray(Wv, np.float32)
    q = np.asarray(q, np.float32)
    Wmlp = np.asarray(Wmlp, np.float32)
    Wb = np.asarray(Wb, np.float32)
    a = float(np.log1p(np.exp(np.float64(np.asarray(alpha)))))
    la2 = L * a / 2.0

    heads, dh = q.shape
    wkq = (Wk.reshape(C, heads, dh) * q[None]).sum(-1) / np.sqrt(dh)
    wkq = np.ascontiguousarray(wkq, np.float32)                      # [C, 2]
    mh = np.zeros((C, heads, 4), np.float32)
    for h in range(heads):
        mh[:, h, :] = Wv[:, h * dh:(h + 1) * dh] @ Wmlp[h * dh:(h + 1) * dh, :]
    mh = np.ascontiguousarray(mh.reshape(C, 8))                      # [C, 8]
    wbc = np.ascontiguousarray(
        Wb.transpose(1, 2, 0).reshape(C, C * 4), np.float32)         # [C, (d k)]
    ident = np.eye(C, dtype=np.float32)

    nc = build(la2)
    xs = x.reshape(B, L, C)
    in_maps = []
    for i in range(NCORES):
        in_maps.append({
            "x": np.ascontiguousarray(
                xs[i * BLOC:(i + 1) * BLOC].reshape(BLOC * L, C)),
            "wkq": wkq, "mh": mh, "wb": wbc, "ident": ident,
        })
    res = run_bass_kernel_spmd(nc, in_maps, core_ids=list(range(NCORES)))
    outs = [res.results[i]["out"].reshape(BLOC, H, W, C) for i in range(NCORES)]
    return np.concatenate(outs, axis=0)
